# revision 12
# baseline (speedup 1.0000x reference)
"""Trainium2 Bass kernel for the nn_Decoder LSTM-decoder problem.

Reference computation (per agent, 12 steps):
    gates = dec_in @ w_ih.T + h @ w_hh.T + (b_ih + b_hh)
    i, f, g, o = split(gates); c = sig(f)*c + sig(i)*tanh(g); h = sig(o)*tanh(c)
    rel = h @ w_hp.T + b_hp; dec_in = rel @ w_se.T + b_se
Output: rel per step, [12, N, 2].

Key algebraic fusion: dec_in_t is a linear function of h_t, so for steps >= 2
    gates_t = h_{t-1} @ W_eff.T + b_eff,  W_eff = w_hh + w_ih @ w_se @ w_hp
and step 1 uses w_hh plus U = w_ih @ w_se applied to last_pos_rel.
last_pos is dead (never affects the output).

Distribution: pure data parallel over the agent axis, 8192 agents per core
on 8 NeuronCores; weights replicated.

On-chip layout: [feature partitions, agent free]. h0/c0/last_pos_rel are
transposed + cast to bf16 on the host so they DMA straight into the
[128, 8192] SBUF state tiles (no on-chip transposes or casts).

ScalarE is the bottleneck (5 activation sets per hidden element per step is
inherent to an LSTM; at 1 elem/lane/cycle that is ~454us minimum), so the
schedule is built to keep it streaming: agents are processed in 2048-agent
superpairs and the four gates go sequentially through a 2-slot x 4-bank
PSUM ring, giving every ACTIVATE a 2048-wide free dim (best instruction
overhead amortization that still fits PSUM). tanh(c) of the previous unit
is issued between gate-ACT groups, exactly covering the cross-unit matmul
bubble. The PE is pre-warmed with dummy matmuls during the input DMAs so
step 0 does not run at the cold 1.2 GHz clock.
"""

import sys

if "/opt/trn_rl_repo" not in sys.path:
    sys.path.insert(0, "/opt/trn_rl_repo")

import numpy as np

T = 12          # steps
H = 128         # hidden dim
NCORES = 8
NPC = 8192      # agents per core
CH = 512        # agents per chunk (one PSUM bank at fp32)
SP = 2048       # agents per superpair (one gate tile / ACT)

_CACHE = {}


def _build_program(npc):
    import concourse.bass as bass
    import concourse.tile as tile
    from concourse import bacc, mybir

    dt = mybir.dt
    f32 = dt.float32
    bf16 = dt.bfloat16
    Act = mybir.ActivationFunctionType

    nsp = npc // SP
    assert npc % SP == 0
    nblk = npc // 64   # output partition blocks (64 agents each)

    nc = bacc.Bacc(
        "TRN2",
        target_bir_lowering=False,
        debug=False,
        num_devices=NCORES,
    )

    def din(name, shape, dt_=None):
        return nc.dram_tensor(
            name, list(shape), dt_ or f32, kind="ExternalInput"
        ).ap()

    # state inputs arrive pre-transposed (feature-major) and bf16 from host
    h0t_d = din("h0t", [H, npc], bf16)
    c0t_d = din("c0t", [H, npc], bf16)
    lprt_d = din("lprt", [2, npc], bf16)
    # lhsT layouts, K on partitions. Gate order [i, f, o, g].
    wg_d = din("wg", [H, 4 * H], bf16)    # W_eff.T columns gate-ordered
    whh_d = din("whh", [H, 4 * H], bf16)  # w_hh.T (step 1)
    u_d = din("u", [2, 4 * H], bf16)      # (w_ih @ w_se).T (step 1)
    bias_d = din("bias", [H, 8])          # ACT bias: [b_eff | b1] x [i,f,o,g]
    whp_d = din("whp", [H, 2], bf16)      # w_hp.T
    bhp_d = din("bhp", [2, 1])            # b_hp as per-partition scalar
    out_d = nc.dram_tensor("out", [T, npc, 2], f32, kind="ExternalOutput").ap()

    with tile.TileContext(nc) as tc:
        with (
            tc.tile_pool(name="wpool", bufs=1) as wp,
            tc.tile_pool(name="state", bufs=1) as state,
            tc.tile_pool(name="sig", bufs=2) as sigp,
            tc.tile_pool(name="tmp", bufs=2) as tmpp,
            tc.tile_pool(name="outp", bufs=2) as outp,
            tc.tile_pool(name="ps", bufs=2, space="PSUM") as psp,
        ):
            def wtile(ap, shape, tag, dt_=None):
                t_ = wp.tile(list(shape), dt_ or f32, tag=tag)
                nc.sync.dma_start(t_[:], ap)
                return t_

            # DMA order = step-0 dependency order: whh/u/bias/lpr feed the
            # first gate matmuls; wg/whp aren't needed until later.
            whh = wtile(whh_d, [H, 4 * H], "whh", bf16)
            u = wtile(u_d, [2, 4 * H], "u", bf16)
            bias = wtile(bias_d, [H, 8], "bias")
            bhp = wtile(bhp_d, [2, 1], "bhp")

            h_sb = state.tile([H, npc], bf16, tag="h")
            c_sb = state.tile([H, npc], bf16, tag="c")
            lpr_sb = state.tile([2, npc], bf16, tag="lpr")

            # state inputs go on the scalar/gpsimd DMA queues so they land in
            # parallel with the weight DMAs on the sync queue; chunked so
            # superpair 0's state arrives first.
            nc.scalar.dma_start(lpr_sb[:], lprt_d)
            for p in range(2 * nsp):
                cols = slice(p * (SP // 2), (p + 1) * (SP // 2))
                eng = nc.scalar if p < 2 else nc.gpsimd
                eng.dma_start(h_sb[:, cols], h0t_d[:, cols])
                eng.dma_start(c_sb[:, cols], c0t_d[:, cols])
            wg = wp.tile([H, 4 * H], bf16, tag="wg")
            nc.gpsimd.dma_start(wg[:], wg_d)
            whp = wp.tile([H, 2], bf16, tag="whp")
            nc.gpsimd.dma_start(whp[:], whp_d)

            # HAM warm-up: ~4us of dummy matmuls on the first-arrived weight
            # tile so the PE clock-gate opens before step 0's dense matmuls.
            warm = psp.tile([128, SP], f32, tag="ps", name="warm")
            for _ in range(10):
                nc.tensor.matmul(warm[:, 0:512], whh[:, 0:128],
                                 whh[:, 0:512], start=True, stop=True)
            # touch the sigmoid/tanh table set early so ACT_TABLE_LOAD runs
            # during the input DMAs instead of before the first real ACT
            tldt = tmpp.tile([1, 1], f32, tag="tld", bufs=1)
            nc.scalar.activation(tldt[:], bias[0:1, 0:1],
                                 Act.Sigmoid)

            def front(t, P):
                """Sequential gates through the PSUM ring + cell update."""
                first = t == 0
                W = whh if first else wg
                bcol = 4 if first else 0
                cols = slice(P * SP, (P + 1) * SP)
                c_pr = c_sb[:, cols]

                def gate_mm(g):
                    gt = psp.tile([128, SP], f32, tag="ps", name=f"gt{g}")
                    wsl = slice(g * H, (g + 1) * H)
                    for q in range(SP // CH):
                        hs = slice(P * SP + q * CH, P * SP + (q + 1) * CH)
                        osl = slice(q * CH, (q + 1) * CH)
                        if first:
                            nc.tensor.matmul(
                                gt[:, osl], u[:, wsl], lpr_sb[:, hs],
                                start=True, stop=False)
                        nc.tensor.matmul(
                            gt[:, osl], W[:, wsl], h_sb[:, hs],
                            start=not first, stop=True)
                    return gt

                def act(gt, dst, fn, col):
                    nc.scalar.activation(dst[:], gt[:], fn,
                                         bias=bias[:, col:col + 1])

                # gate order [i, f, g, o]; column groups in W are [i, f, o, g]
                si = sigp.tile([128, SP], bf16, tag="si")
                act(gate_mm(0), si, Act.Sigmoid, bcol)
                sf = sigp.tile([128, SP], bf16, tag="sf")
                act(gate_mm(1), sf, Act.Sigmoid, bcol + 1)
                tg = sigp.tile([128, SP], bf16, tag="tg")
                act(gate_mm(3), tg, Act.Tanh, bcol + 3)
                m1 = tmpp.tile([128, SP], bf16, tag="m1")
                nc.vector.tensor_mul(m1[:], sf[:], c_pr)
                so = sigp.tile([128, SP], bf16, tag="so")
                act(gate_mm(2), so, Act.Sigmoid, bcol + 2)
                m2 = tmpp.tile([128, SP], bf16, tag="m2")
                nc.vector.tensor_mul(m2[:], si[:], tg[:])
                nc.vector.tensor_add(c_pr, m1[:], m2[:])
                return so

            def back(t, P, so):
                """tanh(c) + h update (deferred one unit)."""
                cols = slice(P * SP, (P + 1) * SP)
                tcl = sigp.tile([128, SP], bf16, tag="tc")
                nc.scalar.activation(tcl[:], c_sb[:, cols], Act.Tanh)
                nc.vector.tensor_mul(h_sb[:, cols], so[:], tcl[:])

            def rel_sp(t, P, xyb):
                """rel = w_hp @ h + b_hp, evacuated with fused bias-add
                (deferred two units)."""
                rp = psp.tile([2, SP], f32, tag="ps", name="rel")
                for q in range(SP // CH):
                    hs = slice(P * SP + q * CH, P * SP + (q + 1) * CH)
                    osl = slice(q * CH, (q + 1) * CH)
                    nc.tensor.matmul(
                        rp[0:2, osl], whp[:], h_sb[:, hs],
                        start=True, stop=True)
                ex = tmpp.tile([2, SP], f32, tag="ex")
                nc.vector.tensor_scalar_add(ex[:], rp[:], bhp[0:2, 0:1])
                prt = slice(32 * P, 32 * (P + 1))
                nc.sync.dma_start(xyb[prt, 0:64], ex[0:1, :])
                nc.sync.dma_start(xyb[prt, 64:128], ex[1:2, :])

            def flush_rows(t, xyb, rlo, rhi):
                relpk = outp.tile([nblk, 128], f32, tag="relpk")
                rv = relpk[:].rearrange("q (a k) -> q a k", k=2)
                rs = slice(rlo, rhi)
                nc.vector.tensor_copy(rv[rs, :, 0], xyb[rs, 0:64])
                nc.vector.tensor_copy(rv[rs, :, 1], xyb[rs, 64:128])
                nc.sync.dma_start(
                    out_d[t, rlo * 64:rhi * 64].rearrange(
                        "(q a) k -> q (a k)", a=64), relpk[rs, :])

            # ---- unit pipeline: FRONT(k) | BACK(k-1) | REL(k-2) ----
            units = [(t, P) for t in range(T) for P in range(nsp)]
            K = len(units)
            so_of = {}
            blks = {}
            done_sp = {t: 0 for t in range(T)}

            def get_blk(t):
                if t not in blks:
                    blks[t] = outp.tile([nblk, 128], f32, tag="xyb",
                                        name=f"xyb{t}")
                return blks[t]

            def emit_rel(kk):
                t, P = units[kk]
                rel_sp(t, P, get_blk(t))
                done_sp[t] += 1
                if t == T - 1:
                    # incremental flush on the last step to shorten the tail
                    flush_rows(t, get_blk(t), 32 * P, 32 * (P + 1))
                    if done_sp[t] == nsp:
                        blks.pop(t)
                elif done_sp[t] == nsp:
                    flush_rows(t, blks.pop(t), 0, nblk)

            for k, (t, P) in enumerate(units):
                so_of[k] = front(t, P)
                if k >= 1:
                    ta, pa = units[k - 1]
                    back(ta, pa, so_of.pop(k - 1))
                if k >= 2:
                    emit_rel(k - 2)
            emit_rel(K - 2)
            back(*units[K - 1], so_of.pop(K - 1))
            emit_rel(K - 1)
            assert not so_of and not blks

    nc.compile()
    return nc


def _fold_weights(w_ih, w_hh, b_ih, b_hh, w_se, b_se, w_hp, b_hp):
    """Host-side constant folding. Gate order [i, f, o, g] (torch order in
    the 4H rows is i, f, g, o)."""
    import ml_dtypes
    mf = ml_dtypes.bfloat16

    perm = np.concatenate([
        np.arange(0, H), np.arange(H, 2 * H),
        np.arange(3 * H, 4 * H), np.arange(2 * H, 3 * H),
    ])
    W_eff = w_hh + w_ih @ w_se @ w_hp                      # [4H, H]
    b_eff = (b_hp @ w_se.T + b_se) @ w_ih.T + b_ih + b_hh  # [4H]
    U = w_ih @ w_se                                        # [4H, 2]
    b1 = b_se @ w_ih.T + b_ih + b_hh                       # [4H]

    Wp, bp = W_eff[perm], b_eff[perm]
    Whhp, Up, b1p = w_hh[perm], U[perm], b1[perm]
    f = np.float32
    bias = np.stack([bp[0:H], bp[H:2*H], bp[2*H:3*H], bp[3*H:4*H],
                     b1p[0:H], b1p[H:2*H], b1p[2*H:3*H], b1p[3*H:4*H]],
                    axis=1)  # [H, 8]
    return {
        "wg": np.ascontiguousarray(Wp.T.astype(mf)),
        "whh": np.ascontiguousarray(Whhp.T.astype(mf)),
        "u": np.ascontiguousarray(Up.T.astype(mf)),
        "bias": np.ascontiguousarray(bias, f),
        "whp": np.ascontiguousarray(w_hp.T.astype(mf)),
        "bhp": np.ascontiguousarray(b_hp.reshape(2, 1), f),
    }


def kernel(last_pos, last_pos_rel, h0, c0,
           w_ih, w_hh, b_ih, b_hh, w_se, b_se, w_hp, b_hp):
    import ml_dtypes
    mf = ml_dtypes.bfloat16

    last_pos_rel = np.asarray(last_pos_rel, np.float32)
    h0 = np.asarray(h0, np.float32)
    c0 = np.asarray(c0, np.float32)
    consts = _fold_weights(
        np.asarray(w_ih, np.float32), np.asarray(w_hh, np.float32),
        np.asarray(b_ih, np.float32), np.asarray(b_hh, np.float32),
        np.asarray(w_se, np.float32), np.asarray(b_se, np.float32),
        np.asarray(w_hp, np.float32), np.asarray(b_hp, np.float32),
    )

    npeds = h0.shape[0]
    npc = npeds // NCORES
    if "nc" not in _CACHE or _CACHE.get("npc") != npc:
        _CACHE["nc"] = _build_program(npc)
        _CACHE["npc"] = npc
    nc = _CACHE["nc"]

    # feature-major bf16 state uploads (transpose + cast on host)
    h0t = np.ascontiguousarray(h0.T.astype(mf))          # [H, N]
    c0t = np.ascontiguousarray(c0.T.astype(mf))
    lprt = np.ascontiguousarray(last_pos_rel.T.astype(mf))  # [2, N]

    in_maps = []
    for ci in range(NCORES):
        cols = slice(ci * npc, (ci + 1) * npc)
        m = {"h0t": h0t[:, cols], "c0t": c0t[:, cols], "lprt": lprt[:, cols]}
        m.update(consts)
        in_maps.append(m)

    from concourse.bass_utils import run_bass_kernel_spmd
    import os

    res = run_bass_kernel_spmd(
        nc, in_maps, list(range(NCORES)),
        tmpdir=os.environ.get("KERNEL_TRACE_DIR"),
    )
    _CACHE["exec_time_ns"] = res.exec_time_ns
    _CACHE["results"] = res
    outs = [np.asarray(res.results[i]["out"]) for i in range(NCORES)]
    return np.concatenate(outs, axis=1)
